# revision 7
# baseline (speedup 1.0000x reference)
"""CTC aligner kernel for Trainium2 (8 NeuronCores, data-parallel over batch).

The reference's forward/backward recursions collapse: in each scan step the
logsumexp factors out (f_prev has no s-dependence in the added term), so with
q[t] = logits[t] + trans[t-1] (+b2), q[0] = f0, q[T] = 0:
    alignments[t] = softmax_s(q[t] + q[t+1])
The only heavy compute left is the pair MLP (trans), a batched matmul.

Per core (4 examples): frames are DMA-xbar-transposed (bf16) into [h, t]
layout, MM1 (frames @ W1, K=512 contracted as 4x128 over two shifted time
views) gives hidden^T in PSUM, relu+b1 writes hidden^T to SBUF bf16, MM2
(hidden slices stationary, W2 moving) gives trans in [t, s] layout per
128-row group, b2 is injected via a K=1 ones-matmul that also initializes
PSUM. Pointwise phase builds q, a partition-shifted copy of q via SBUF->SBUF
DMA, one add for the scores, then exp / row-sum / reciprocal / scale.
"""

import numpy as np
import ml_dtypes

B, T, S, H = 32, 2000, 63, 256
NCORES = 8
BPC = B // NCORES  # examples per core
TPAD = 2048        # t padded to 16*128
NG = TPAD // 128   # 16 groups of 128 timesteps

_built = {}


def _build():
    from concourse import mybir, tile, bacc

    f32 = mybir.dt.float32
    f32r = mybir.dt.float32r
    bf16 = mybir.dt.bfloat16
    Alu = mybir.AluOpType
    Act = mybir.ActivationFunctionType

    nc = bacc.Bacc("TRN2", target_bir_lowering=False, debug=False,
                   num_devices=NCORES)

    frames_bf = nc.dram_tensor("frames_bf", [BPC, TPAD, H], bf16,
                               kind="ExternalInput").ap()
    logits = nc.dram_tensor("logits", [BPC, TPAD, S], f32,
                            kind="ExternalInput").ap()
    f0 = nc.dram_tensor("f0", [BPC, S], f32, kind="ExternalInput").ap()
    w1 = nc.dram_tensor("W1", [2 * H, H], bf16, kind="ExternalInput").ap()
    w2 = nc.dram_tensor("W2", [H, S], bf16, kind="ExternalInput").ap()
    b1 = nc.dram_tensor("b1", [H, 1], f32, kind="ExternalInput").ap()
    b2r = nc.dram_tensor("b2r", [1, 8 * S], bf16, kind="ExternalInput").ap()
    out = nc.dram_tensor("out", [BPC, TPAD, S], f32,
                         kind="ExternalOutput").ap()

    # MM1 time chunks over the 1999 pair indices
    CHUNKS = [(0, 512), (512, 512), (1024, 512), (1536, 463)]
    TLAST = T - 15 * 128  # 80 valid rows in the final group

    with tile.TileContext(nc) as tc:
        with (
            tc.tile_pool(name="const", bufs=1) as constp,
            tc.tile_pool(name="frames", bufs=2) as framesp,
            tc.tile_pool(name="hid", bufs=2) as hidp,
            tc.tile_pool(name="mm1ps", bufs=2, space="PSUM") as mm1p,
            tc.tile_pool(name="mm2ps", bufs=2, space="PSUM") as mm2p,
            tc.tile_pool(name="pw", bufs=2) as pwp,
        ):
            w1sb = constp.tile([128, 4, H], bf16)
            nc.sync.dma_start(w1sb[:], w1.rearrange("(k p) j -> p k j", p=128))
            w2sb = constp.tile([128, 2, S], bf16)
            nc.sync.dma_start(w2sb[:], w2.rearrange("(k p) s -> p k s", p=128))
            b1sb = constp.tile([128, 2], f32)
            nc.sync.dma_start(b1sb[:], b1.rearrange("(j p) o -> p (j o)", p=128))
            b2sb = constp.tile([1, 8 * S], bf16)
            nc.sync.dma_start(b2sb[:], b2r[:])
            ones = constp.tile([1, 128], bf16)
            nc.vector.memset(ones[:], 1.0)

            for b in range(BPC):
                # frames^T in SBUF: ft[p, h, t] = frames[b, t, 128*h + p]
                ft = framesp.tile([128, 2, TPAD], bf16, tag="ft")
                for h in range(2):
                    nc.sync.dma_start(ft[:, h], frames_bf[b, :, 128 * h:128 * h + 128],
                                      transpose=True)

                # hidden^T: hid[p, j, 1 + pair] = relu(pair MLP)[128*j + p]
                hid = hidp.tile([128, 2, TPAD], bf16, tag="hid")
                nc.gpsimd.memset(hid[:, :, 0:1], 0.0)
                nc.gpsimd.memset(hid[:, :, 1 + 1999:], 0.0)
                for ci, (t0, w) in enumerate(CHUNKS):
                    for j in range(2):
                        ps = mm1p.tile([128, 512], f32, tag="mm1")
                        for k in range(4):
                            half = k if k < 2 else k - 2
                            toff = t0 + 1 if k < 2 else t0
                            nc.tensor.matmul(
                                ps[:, :w],
                                w1sb[:, k, 128 * j:128 * j + 128],
                                ft[:, half, toff:toff + w],
                                start=(k == 0), stop=(k == 3),
                            )
                        dst = hid[:, j, 1 + t0:1 + t0 + w]
                        if ci == 0:
                            nc.vector.tensor_scalar(
                                dst, ps[:, :w], b1sb[:, j:j + 1], 0.0,
                                op0=Alu.add, op1=Alu.max)
                        else:
                            nc.scalar.activation(dst, ps[:, :w], Act.Relu,
                                                 bias=b1sb[:, j:j + 1])

                # logits tile: lt[p, g, s] = logits[b, 128*g + p, s]
                lt = pwp.tile([128, NG, S], f32, tag="L")
                nc.sync.dma_start(lt[:], logits[b].rearrange("(g p) s -> p g s", p=128))

                # q[p, g] = logits[t] + trans[t-1] + b2, t = 128*g + p
                # q[:, NG] is an all-zero extra group for the shifted view.
                q = pwp.tile([128, NG + 1, S], f32, tag="q")
                nc.gpsimd.memset(q[:, NG], 0.0)
                for st in range(2):
                    ps2 = mm2p.tile([128, 8, S], f32, tag="mm2")
                    ps2f = ps2[:].rearrange("p g s -> p (g s)")
                    # K=1 ones-matmul: adds b2 everywhere and initializes PSUM.
                    nc.tensor.matmul(ps2f, ones[0:1, :], b2sb[:],
                                     start=True, stop=False)
                    for gl in range(8):
                        g = 8 * st + gl
                        for k in range(2):
                            nc.tensor.matmul(
                                ps2[:, gl],
                                hid[:, k, 128 * g:128 * g + 128],
                                w2sb[:, k],
                                start=False, stop=(gl == 7 and k == 1),
                            )
                    nc.vector.tensor_add(q[:, 8 * st:8 * st + 8],
                                         lt[:, 8 * st:8 * st + 8], ps2[:])
                # overwrite q[t=0] with f0 (one-hot-ish initial state row)
                nc.sync.dma_start(q[0:1, 0], f0[b:b + 1, :])
                # zero q rows past t=1999 in the last group (they hold
                # pad-garbage + b2; q[2000] must be 0 for the final score).
                # logits pad rows are zeros in DRAM - reuse them.
                nc.sync.dma_start(q[TLAST:128, NG - 1],
                                  logits[b, T:T + (128 - TLAST), :])

                # partition-shifted view: qsh[p, g] = q at t = 128*g + p + 1
                qsh = pwp.tile([128, NG, S], f32, tag="qsh")
                nc.sync.dma_start(qsh[0:127], q[1:128, 0:NG])
                nc.sync.dma_start(qsh[127:128], q[0:1, 1:NG + 1])

                sc = pwp.tile([128, NG, S], f32, tag="sc")
                nc.vector.tensor_add(sc[:], q[:, 0:NG], qsh[:])
                e = pwp.tile([128, NG, S], f32, tag="e")
                nc.scalar.activation(e[:], sc[:], Act.Exp)
                sums = pwp.tile([128, NG], f32, tag="sums")
                nc.vector.reduce_sum(sums[:], e[:], mybir.AxisListType.X)
                rec = pwp.tile([128, NG], f32, tag="rec")
                nc.vector.reciprocal(rec[:], sums[:])
                r = pwp.tile([128, NG, S], f32, tag="r")
                for g in range(NG):
                    if g % 4 == 0:
                        nc.vector.tensor_scalar_mul(r[:, g], e[:, g],
                                                    rec[:, g:g + 1])
                    else:
                        nc.scalar.mul(r[:, g], e[:, g], rec[:, g:g + 1])
                nc.sync.dma_start(out[b].rearrange("(g p) s -> p g s", p=128), r[:])

    nc.compile()
    return nc


def _get_nc():
    if "nc" not in _built:
        _built["nc"] = _build()
    return _built["nc"]


def prep_in_maps(ctc_logits, frame_embeddings, keyword_tokens, W1, b1, W2, b2):
    ctc_logits = np.asarray(ctc_logits, dtype=np.float32)
    frame_embeddings = np.asarray(frame_embeddings, dtype=np.float32)
    keyword_tokens = np.asarray(keyword_tokens)
    W1 = np.asarray(W1, dtype=np.float32)
    b1 = np.asarray(b1, dtype=np.float32)
    W2 = np.asarray(W2, dtype=np.float32)
    b2 = np.asarray(b2, dtype=np.float32)

    # f0[b, s] = (s == 0) * log_softmax(logits[b, 0])[kw[b, 0]]
    lg0 = ctc_logits[:, 0, :].astype(np.float64)
    m = lg0.max(axis=-1)
    lse = m + np.log(np.exp(lg0 - m[:, None]).sum(axis=-1))
    kw0 = keyword_tokens[:, 0].astype(np.int64)
    f0 = np.zeros((B, S), np.float32)
    f0[:, 0] = (lg0[np.arange(B), kw0] - lse).astype(np.float32)

    frames_bf = np.zeros((B, TPAD, H), ml_dtypes.bfloat16)
    frames_bf[:, :T] = frame_embeddings.astype(ml_dtypes.bfloat16)
    logits_pad = np.zeros((B, TPAD, S), np.float32)
    logits_pad[:, :T] = ctc_logits

    w1b = np.ascontiguousarray(W1).astype(ml_dtypes.bfloat16)
    w2b = np.ascontiguousarray(W2).astype(ml_dtypes.bfloat16)
    b1c = np.ascontiguousarray(b1.reshape(H, 1), dtype=np.float32)
    b2r = np.ascontiguousarray(np.tile(b2, 8).reshape(1, 8 * S)).astype(
        ml_dtypes.bfloat16)

    in_maps = []
    for c in range(NCORES):
        sl = slice(BPC * c, BPC * (c + 1))
        in_maps.append({
            "frames_bf": np.ascontiguousarray(frames_bf[sl]),
            "logits": np.ascontiguousarray(logits_pad[sl]),
            "f0": np.ascontiguousarray(f0[sl]),
            "W1": w1b, "W2": w2b, "b1": b1c, "b2r": b2r,
        })
    return in_maps


def kernel(ctc_logits, frame_embeddings, keyword_tokens, W1, b1, W2, b2):
    from concourse.bass_utils import run_bass_kernel_spmd

    in_maps = prep_in_maps(ctc_logits, frame_embeddings, keyword_tokens,
                           W1, b1, W2, b2)
    nc = _get_nc()
    res = run_bass_kernel_spmd(nc, in_maps, list(range(NCORES)))
    out = np.concatenate([res.results[c]["out"][:, :T, :]
                          for c in range(NCORES)], axis=0)
    return np.ascontiguousarray(out, dtype=np.float32)


# revision 9
# speedup vs baseline: 1.0377x; 1.0377x over previous
"""CTC aligner kernel for Trainium2 (8 NeuronCores, data-parallel over batch).

The reference's forward/backward recursions collapse: in each scan step the
logsumexp factors out (f_prev has no s-dependence in the added term), so with
q[t] = logits[t] + trans[t-1] (+b2), q[0] = f0, q[T] = 0:
    alignments[t] = softmax_s(q[t] + q[t+1])
The only heavy compute left is the pair MLP (trans), a batched matmul.

Per core (4 examples): frames are DMA-xbar-transposed (bf16) into [h, t]
layout, MM1 (frames @ W1, K=512 contracted as 4x128 over two shifted time
views) gives hidden^T in PSUM, relu+b1 writes hidden^T to SBUF bf16, MM2
(hidden slices stationary, W2 moving) gives trans in [t, s] layout per
128-row group, b2 is injected via a K=1 ones-matmul that also initializes
PSUM. Pointwise phase builds q, a partition-shifted copy of q via SBUF->SBUF
DMA, one add for the scores, then exp / row-sum / reciprocal / scale.

Logits and the output use a host-pretiled (p, g, s) layout so every big DMA
is contiguous per partition (128 descriptors instead of 2048). Emission is
phase-major (all MM1 matmuls back to back, then the MM2/pointwise blocks)
to keep the PE dense and HAM-warm.
"""

import numpy as np
import ml_dtypes

B, T, S, H = 32, 2000, 63, 256
NCORES = 8
BPC = B // NCORES  # examples per core
TPAD = 2048        # t padded to 16*128
NG = TPAD // 128   # 16 groups of 128 timesteps
TLAST = T - 15 * 128  # 80 valid rows in the final group

_built = {}


def _build():
    from concourse import mybir, tile, bacc

    f32 = mybir.dt.float32
    bf16 = mybir.dt.bfloat16
    Alu = mybir.AluOpType
    Act = mybir.ActivationFunctionType

    nc = bacc.Bacc("TRN2", target_bir_lowering=False, debug=False,
                   num_devices=NCORES)

    frames_bf = nc.dram_tensor("frames_bf", [BPC, TPAD, H], bf16,
                               kind="ExternalInput").ap()
    # host-pretiled: logits_t[b, p, g, s] = logits[b, 128*g + p, s]
    logits_t = nc.dram_tensor("logits_t", [BPC, 128, NG, S], f32,
                              kind="ExternalInput").ap()
    zeros48 = nc.dram_tensor("zeros48", [128 - TLAST, S], f32,
                             kind="ExternalInput").ap()
    f0 = nc.dram_tensor("f0", [BPC, S], f32, kind="ExternalInput").ap()
    w1 = nc.dram_tensor("W1", [2 * H, H], bf16, kind="ExternalInput").ap()
    w2 = nc.dram_tensor("W2", [H, S], bf16, kind="ExternalInput").ap()
    b1 = nc.dram_tensor("b1", [H, 1], f32, kind="ExternalInput").ap()
    b2r = nc.dram_tensor("b2r", [1, 8 * S], bf16, kind="ExternalInput").ap()
    # host un-tiles: out[b, 128*g + p, s] = out_t[b, p, g, s]
    out_t = nc.dram_tensor("out_t", [BPC, 128, NG, S], f32,
                           kind="ExternalOutput").ap()

    # MM1 time chunks over the 1999 pair indices
    CHUNKS = [(0, 512), (512, 512), (1024, 512), (1536, 463)]

    with tile.TileContext(nc) as tc:
        with (
            tc.tile_pool(name="const", bufs=1) as constp,
            tc.tile_pool(name="frames", bufs=1) as framesp,
            tc.tile_pool(name="hid", bufs=1) as hidp,
            tc.tile_pool(name="mm1ps", bufs=3, space="PSUM") as mm1p,
            tc.tile_pool(name="mm2ps", bufs=3, space="PSUM") as mm2p,
            tc.tile_pool(name="pw", bufs=2) as pwp,
        ):
            w1sb = constp.tile([128, 4, H], bf16)
            nc.sync.dma_start(w1sb[:], w1.rearrange("(k p) j -> p k j", p=128))
            w2sb = constp.tile([128, 2, S], bf16)
            nc.sync.dma_start(w2sb[:], w2.rearrange("(k p) s -> p k s", p=128))
            b1sb = constp.tile([128, 2], f32)
            nc.sync.dma_start(b1sb[:], b1.rearrange("(j p) o -> p (j o)", p=128))
            b2sb = constp.tile([1, 8 * S], bf16)
            nc.sync.dma_start(b2sb[:], b2r[:])
            ones = constp.tile([1, 128], bf16)
            nc.vector.memset(ones[:], 1.0)

            # ---- phase 0: transpose-load all frames ----
            fts = []
            for b in range(BPC):
                ft = framesp.tile([128, 2, TPAD], bf16, tag=f"ft{b}")
                for h in range(2):
                    nc.sync.dma_start(ft[:, h],
                                      frames_bf[b, :, 128 * h:128 * h + 128],
                                      transpose=True)
                fts.append(ft)

            # ---- phase 1: all MM1 matmuls (dense on PE) ----
            hids = []
            for b in range(BPC):
                hid = hidp.tile([128, 2, TPAD], bf16, tag=f"hid{b}")
                nc.gpsimd.memset(hid[:, :, 0:1], 0.0)
                nc.gpsimd.memset(hid[:, :, 1 + 1999:], 0.0)
                hids.append(hid)
            for b in range(BPC):
                ft, hid = fts[b], hids[b]
                for ci, (t0, w) in enumerate(CHUNKS):
                    for j in range(2):
                        ps = mm1p.tile([128, 512], f32, tag="mm1")
                        for k in range(4):
                            half = k if k < 2 else k - 2
                            toff = t0 + 1 if k < 2 else t0
                            nc.tensor.matmul(
                                ps[:, :w],
                                w1sb[:, k, 128 * j:128 * j + 128],
                                ft[:, half, toff:toff + w],
                                start=(k == 0), stop=(k == 3),
                            )
                        dst = hid[:, j, 1 + t0:1 + t0 + w]
                        if ci % 2 == 0 and j == 0:
                            nc.vector.tensor_scalar(
                                dst, ps[:, :w], b1sb[:, j:j + 1], 0.0,
                                op0=Alu.add, op1=Alu.max)
                        else:
                            nc.scalar.activation(dst, ps[:, :w], Act.Relu,
                                                 bias=b1sb[:, j:j + 1])

            # ---- phase 2: per example MM2 + pointwise softmax ----
            for b in range(BPC):
                hid = hids[b]
                lt = pwp.tile([128, NG, S], f32, tag="L")
                nc.sync.dma_start(lt[:], logits_t[b])

                q = pwp.tile([128, NG + 1, S], f32, tag="q")
                nc.gpsimd.memset(q[:, NG], 0.0)
                for st in range(2):
                    ps2 = mm2p.tile([128, 8, S], f32, tag="mm2")
                    ps2f = ps2[:].rearrange("p g s -> p (g s)")
                    # K=1 ones-matmul: adds b2 everywhere, initializes PSUM
                    nc.tensor.matmul(ps2f, ones[0:1, :], b2sb[:],
                                     start=True, stop=False)
                    for gl in range(8):
                        g = 8 * st + gl
                        for k in range(2):
                            nc.tensor.matmul(
                                ps2[:, gl],
                                hid[:, k, 128 * g:128 * g + 128],
                                w2sb[:, k],
                                start=False, stop=(gl == 7 and k == 1),
                            )
                    nc.vector.tensor_add(q[:, 8 * st:8 * st + 8],
                                         lt[:, 8 * st:8 * st + 8], ps2[:])
                # q[t=0] = f0; q rows past t=1999 (pad garbage + b2) = 0
                nc.gpsimd.dma_start(q[0:1, 0], f0[b:b + 1, :])
                nc.gpsimd.dma_start(q[TLAST:128, NG - 1], zeros48[:])

                # partition-shifted view: qsh[p, g] = q at t = 128*g + p + 1
                qsh = pwp.tile([128, NG, S], f32, tag="qsh")
                nc.gpsimd.dma_start(qsh[0:127], q[1:128, 0:NG])
                nc.gpsimd.dma_start(qsh[127:128], q[0:1, 1:NG + 1])

                sc = pwp.tile([128, NG, S], f32, tag="sc")
                nc.vector.tensor_add(sc[:], q[:, 0:NG], qsh[:])
                e = pwp.tile([128, NG, S], f32, tag="e")
                nc.scalar.activation(e[:], sc[:], Act.Exp)
                sums = pwp.tile([128, NG], f32, tag="sums")
                nc.vector.reduce_sum(sums[:], e[:], mybir.AxisListType.X)
                rec = pwp.tile([128, NG], f32, tag="rec")
                nc.vector.reciprocal(rec[:], sums[:])
                r = pwp.tile([128, NG, S], f32, tag="r")
                nc.vector.tensor_mul(r[:, 0:8], e[:, 0:8],
                                     rec[:, 0:8].broadcast_to([128, 8, S]))
                for g in range(8, NG):
                    nc.scalar.mul(r[:, g], e[:, g], rec[:, g:g + 1])
                nc.sync.dma_start(out_t[b], r[:])

    nc.compile()
    return nc


def _get_nc():
    if "nc" not in _built:
        _built["nc"] = _build()
    return _built["nc"]


def prep_in_maps(ctc_logits, frame_embeddings, keyword_tokens, W1, b1, W2, b2):
    ctc_logits = np.asarray(ctc_logits, dtype=np.float32)
    frame_embeddings = np.asarray(frame_embeddings, dtype=np.float32)
    keyword_tokens = np.asarray(keyword_tokens)
    W1 = np.asarray(W1, dtype=np.float32)
    b1 = np.asarray(b1, dtype=np.float32)
    W2 = np.asarray(W2, dtype=np.float32)
    b2 = np.asarray(b2, dtype=np.float32)

    # f0[b, s] = (s == 0) * log_softmax(logits[b, 0])[kw[b, 0]]
    lg0 = ctc_logits[:, 0, :].astype(np.float64)
    m = lg0.max(axis=-1)
    lse = m + np.log(np.exp(lg0 - m[:, None]).sum(axis=-1))
    kw0 = keyword_tokens[:, 0].astype(np.int64)
    f0 = np.zeros((B, S), np.float32)
    f0[:, 0] = (lg0[np.arange(B), kw0] - lse).astype(np.float32)

    frames_bf = np.zeros((B, TPAD, H), ml_dtypes.bfloat16)
    frames_bf[:, :T] = frame_embeddings.astype(ml_dtypes.bfloat16)
    # tiled layout: logits_t[b, p, g, s] = logits[b, 128*g + p, s] (pad t with 0)
    logits_pad = np.zeros((B, TPAD, S), np.float32)
    logits_pad[:, :T] = ctc_logits
    logits_tl = np.ascontiguousarray(
        logits_pad.reshape(B, NG, 128, S).transpose(0, 2, 1, 3))

    w1b = np.ascontiguousarray(W1).astype(ml_dtypes.bfloat16)
    w2b = np.ascontiguousarray(W2).astype(ml_dtypes.bfloat16)
    b1c = np.ascontiguousarray(b1.reshape(H, 1), dtype=np.float32)
    b2r = np.ascontiguousarray(np.tile(b2, 8).reshape(1, 8 * S)).astype(
        ml_dtypes.bfloat16)
    zeros48 = np.zeros((128 - TLAST, S), np.float32)

    in_maps = []
    for c in range(NCORES):
        sl = slice(BPC * c, BPC * (c + 1))
        in_maps.append({
            "frames_bf": np.ascontiguousarray(frames_bf[sl]),
            "logits_t": np.ascontiguousarray(logits_tl[sl]),
            "f0": np.ascontiguousarray(f0[sl]),
            "zeros48": zeros48,
            "W1": w1b, "W2": w2b, "b1": b1c, "b2r": b2r,
        })
    return in_maps


def untile_out(res_out):
    # res_out: (BPC, 128, NG, S) -> (BPC, T, S)
    return res_out.transpose(0, 2, 1, 3).reshape(BPC, TPAD, S)[:, :T, :]


def kernel(ctc_logits, frame_embeddings, keyword_tokens, W1, b1, W2, b2):
    from concourse.bass_utils import run_bass_kernel_spmd

    in_maps = prep_in_maps(ctc_logits, frame_embeddings, keyword_tokens,
                           W1, b1, W2, b2)
    nc = _get_nc()
    res = run_bass_kernel_spmd(nc, in_maps, list(range(NCORES)))
    out = np.concatenate([untile_out(res.results[c]["out_t"])
                          for c in range(NCORES)], axis=0)
    return np.ascontiguousarray(out, dtype=np.float32)


# revision 11
# speedup vs baseline: 1.1371x; 1.0958x over previous
"""CTC aligner kernel for Trainium2 (8 NeuronCores, data-parallel over batch).

The reference's forward/backward recursions collapse: in each scan step the
logsumexp factors out (f_prev has no s-dependence in the added term), so with
q[t] = logits[t] + trans[t-1] (+b2), q[0] = f0, q[T] = 0:
    alignments[t] = softmax_s(q[t] + q[t+1])
The only heavy compute left is the pair MLP (trans), a batched matmul.

Per core (4 examples): frames are DMA-xbar-transposed (bf16) into [h, t]
layout, MM1 (frames @ W1, K=512 contracted as 4x128 over two shifted time
views, N=1024 moving) gives hidden^T in PSUM, relu+b1 writes hidden^T to
SBUF bf16, MM2 (hidden slices stationary, W2 moving) gives trans in [t, s]
layout per 128-row group, b2 is injected via a K=1 ones-matmul that also
initializes PSUM. Pointwise phase builds q, a partition-shifted copy of q
via SBUF->SBUF DMA, one add for the scores (the f0 row is patched with a
1-partition add), then exp / row-sum / reciprocal / scale.

Logits and the output use a host-pretiled (p, g, s) layout so every big DMA
is contiguous per partition. Emission is example-major with dense matmul
blocks; pools give cross-example pipelining.
"""

import numpy as np
import ml_dtypes

B, T, S, H = 32, 2000, 63, 256
NCORES = 8
BPC = B // NCORES  # examples per core
TPAD = 2048        # t padded to 16*128
NG = TPAD // 128   # 16 groups of 128 timesteps
TLAST = T - 15 * 128  # 80 valid rows in the final group

_built = {}


def _build():
    from concourse import mybir, tile, bacc

    f32 = mybir.dt.float32
    bf16 = mybir.dt.bfloat16
    Alu = mybir.AluOpType
    Act = mybir.ActivationFunctionType

    nc = bacc.Bacc("TRN2", target_bir_lowering=False, debug=False,
                   num_devices=NCORES)

    frames_bf = nc.dram_tensor("frames_bf", [BPC, TPAD, H], bf16,
                               kind="ExternalInput").ap()
    # host-pretiled: logits_t[b, p, g, s] = logits[b, 128*g + p, s]
    logits_t = nc.dram_tensor("logits_t", [BPC, 128, NG, S], f32,
                              kind="ExternalInput").ap()
    zeros48 = nc.dram_tensor("zeros48", [128 - TLAST, S], f32,
                             kind="ExternalInput").ap()
    # f0r[0, b, s]: f0 rows for all examples on one partition
    f0r = nc.dram_tensor("f0r", [1, BPC, S], f32, kind="ExternalInput").ap()
    w1 = nc.dram_tensor("W1", [2 * H, H], bf16, kind="ExternalInput").ap()
    w2 = nc.dram_tensor("W2", [H, S], bf16, kind="ExternalInput").ap()
    b1 = nc.dram_tensor("b1", [H, 1], f32, kind="ExternalInput").ap()
    b2r = nc.dram_tensor("b2r", [1, 8 * S], bf16, kind="ExternalInput").ap()
    # host un-tiles: out[b, 128*g + p, s] = out_t[b, p, g, s]
    out_t = nc.dram_tensor("out_t", [BPC, 128, NG, S], f32,
                           kind="ExternalOutput").ap()

    # MM1 time chunks over the 1999 pair indices
    CHUNKS = [(0, 512), (512, 512), (1024, 512), (1536, 463)]

    with tile.TileContext(nc) as tc:
        with (
            tc.tile_pool(name="const", bufs=1) as constp,
            tc.tile_pool(name="frames", bufs=1) as framesp,
            tc.tile_pool(name="hid", bufs=1) as hidp,
            tc.tile_pool(name="mm1ps", bufs=3, space="PSUM") as mm1p,
            tc.tile_pool(name="mm2ps", bufs=3, space="PSUM") as mm2p,
            tc.tile_pool(name="pw", bufs=3) as pwp,
        ):
            w1sb = constp.tile([128, 4, H], bf16)
            nc.sync.dma_start(w1sb[:], w1.rearrange("(k p) j -> p k j", p=128))
            w2sb = constp.tile([128, 2, S], bf16)
            nc.sync.dma_start(w2sb[:], w2.rearrange("(k p) s -> p k s", p=128))
            b1sb = constp.tile([128, 2], f32)
            nc.sync.dma_start(b1sb[:], b1.rearrange("(j p) o -> p (j o)", p=128))
            b2sb = constp.tile([1, 8 * S], bf16)
            nc.sync.dma_start(b2sb[:], b2r[:])
            f0sb = constp.tile([1, BPC, S], f32)
            nc.sync.dma_start(f0sb[:], f0r[:])
            ones = constp.tile([1, 128], bf16)
            nc.vector.memset(ones[:], 1.0)

            # ---- transpose-load all frames up front ----
            fts = []
            for b in range(BPC):
                ft = framesp.tile([128, 2, TPAD], bf16, tag=f"ft{b}")
                for h in range(2):
                    nc.sync.dma_start(ft[:, h],
                                      frames_bf[b, :, 128 * h:128 * h + 128],
                                      transpose=True)
                fts.append(ft)

            hids = []
            for b in range(BPC):
                hid = hidp.tile([128, 2, TPAD], bf16, tag=f"hid{b}")
                nc.gpsimd.memset(hid[:, :, 0:1], 0.0)
                nc.gpsimd.memset(hid[:, :, 1 + 1999:], 0.0)
                hids.append(hid)

            for b in range(BPC):
                ft, hid = fts[b], hids[b]
                # ---- MM1: hidden^T = relu(pairs @ W1 + b1) ----
                for ci, (t0, w) in enumerate(CHUNKS):
                    for j in range(2):
                        ps = mm1p.tile([128, 512], f32, tag="mm1")
                        for k in range(4):
                            half = k if k < 2 else k - 2
                            toff = t0 + 1 if k < 2 else t0
                            nc.tensor.matmul(
                                ps[:, :w],
                                w1sb[:, k, 128 * j:128 * j + 128],
                                ft[:, half, toff:toff + w],
                                start=(k == 0), stop=(k == 3),
                            )
                        dst = hid[:, j, 1 + t0:1 + t0 + w]
                        if (ci + j) % 2 == 0:
                            nc.vector.tensor_scalar(
                                dst, ps[:, :w], b1sb[:, j:j + 1], 0.0,
                                op0=Alu.add, op1=Alu.max)
                        else:
                            nc.scalar.activation(dst, ps[:, :w], Act.Relu,
                                                 bias=b1sb[:, j:j + 1])

                # ---- MM2 + q build ----
                lt = pwp.tile([128, NG, S], f32, tag="L")
                nc.sync.dma_start(lt[:], logits_t[b])
                q = pwp.tile([128, NG + 1, S], f32, tag="q")
                nc.gpsimd.memset(q[:, NG], 0.0)
                for st in range(2):
                    ps2 = mm2p.tile([128, 8, S], f32, tag="mm2")
                    ps2f = ps2[:].rearrange("p g s -> p (g s)")
                    # K=1 ones-matmul: adds b2 everywhere, initializes PSUM
                    nc.tensor.matmul(ps2f, ones[0:1, :], b2sb[:],
                                     start=True, stop=False)
                    for gl in range(8):
                        g = 8 * st + gl
                        for k in range(2):
                            nc.tensor.matmul(
                                ps2[:, gl],
                                hid[:, k, 128 * g:128 * g + 128],
                                w2sb[:, k],
                                start=False, stop=(gl == 7 and k == 1),
                            )
                    nc.vector.tensor_add(q[:, 8 * st:8 * st + 8],
                                         lt[:, 8 * st:8 * st + 8], ps2[:])
                # q rows past t=1999 in the last group hold pad-garbage + b2;
                # q[2000] must read as 0 for the final score row.
                nc.sync.dma_start(q[TLAST:128, NG - 1], zeros48[:])

                # ---- pointwise: score = q[t] + q[t+1], softmax over s ----
                qsh = pwp.tile([128, NG, S], f32, tag="qsh")
                nc.sync.dma_start(qsh[0:127], q[1:128, 0:NG])
                nc.sync.dma_start(qsh[127:128], q[0:1, 1:NG + 1])

                sc = pwp.tile([128, NG, S], f32, tag="sc")
                nc.vector.tensor_add(sc[:], q[:, 0:NG], qsh[:])
                # patch t=0: score[0] = f0 + q[1]
                nc.vector.tensor_add(sc[0:1, 0], f0sb[0:1, b], qsh[0:1, 0])
                e = pwp.tile([128, NG, S], f32, tag="e")
                nc.scalar.activation(e[:], sc[:], Act.Exp)
                sums = pwp.tile([128, NG], f32, tag="sums")
                nc.vector.reduce_sum(sums[:], e[:], mybir.AxisListType.X)
                rec = pwp.tile([128, NG], f32, tag="rec")
                nc.vector.reciprocal(rec[:], sums[:])
                r = pwp.tile([128, NG, S], f32, tag="r")
                nc.vector.tensor_mul(r[:, 0:8], e[:, 0:8],
                                     rec[:, 0:8].broadcast_to([128, 8, S]))
                for g in range(8, NG):
                    nc.scalar.mul(r[:, g], e[:, g], rec[:, g:g + 1])
                nc.sync.dma_start(out_t[b], r[:])

    nc.compile()
    return nc


def _get_nc():
    if "nc" not in _built:
        _built["nc"] = _build()
    return _built["nc"]


def prep_in_maps(ctc_logits, frame_embeddings, keyword_tokens, W1, b1, W2, b2):
    ctc_logits = np.asarray(ctc_logits, dtype=np.float32)
    frame_embeddings = np.asarray(frame_embeddings, dtype=np.float32)
    keyword_tokens = np.asarray(keyword_tokens)
    W1 = np.asarray(W1, dtype=np.float32)
    b1 = np.asarray(b1, dtype=np.float32)
    W2 = np.asarray(W2, dtype=np.float32)
    b2 = np.asarray(b2, dtype=np.float32)

    # f0[b, s] = (s == 0) * log_softmax(logits[b, 0])[kw[b, 0]]
    lg0 = ctc_logits[:, 0, :].astype(np.float64)
    m = lg0.max(axis=-1)
    lse = m + np.log(np.exp(lg0 - m[:, None]).sum(axis=-1))
    kw0 = keyword_tokens[:, 0].astype(np.int64)
    f0 = np.zeros((B, S), np.float32)
    f0[:, 0] = (lg0[np.arange(B), kw0] - lse).astype(np.float32)

    frames_bf = np.zeros((B, TPAD, H), ml_dtypes.bfloat16)
    frames_bf[:, :T] = frame_embeddings.astype(ml_dtypes.bfloat16)
    # tiled layout: logits_t[b, p, g, s] = logits[b, 128*g + p, s] (pad t with 0)
    logits_pad = np.zeros((B, TPAD, S), np.float32)
    logits_pad[:, :T] = ctc_logits
    logits_tl = np.ascontiguousarray(
        logits_pad.reshape(B, NG, 128, S).transpose(0, 2, 1, 3))

    w1b = np.ascontiguousarray(W1).astype(ml_dtypes.bfloat16)
    w2b = np.ascontiguousarray(W2).astype(ml_dtypes.bfloat16)
    b1c = np.ascontiguousarray(b1.reshape(H, 1), dtype=np.float32)
    b2r = np.ascontiguousarray(np.tile(b2, 8).reshape(1, 8 * S)).astype(
        ml_dtypes.bfloat16)
    zeros48 = np.zeros((128 - TLAST, S), np.float32)

    in_maps = []
    for c in range(NCORES):
        sl = slice(BPC * c, BPC * (c + 1))
        in_maps.append({
            "frames_bf": np.ascontiguousarray(frames_bf[sl]),
            "logits_t": np.ascontiguousarray(logits_tl[sl]),
            "f0r": np.ascontiguousarray(f0[sl].reshape(1, BPC, S)),
            "zeros48": zeros48,
            "W1": w1b, "W2": w2b, "b1": b1c, "b2r": b2r,
        })
    return in_maps


def untile_out(res_out):
    # res_out: (BPC, 128, NG, S) -> (BPC, T, S)
    return res_out.transpose(0, 2, 1, 3).reshape(BPC, TPAD, S)[:, :T, :]


def kernel(ctc_logits, frame_embeddings, keyword_tokens, W1, b1, W2, b2):
    from concourse.bass_utils import run_bass_kernel_spmd

    in_maps = prep_in_maps(ctc_logits, frame_embeddings, keyword_tokens,
                           W1, b1, W2, b2)
    nc = _get_nc()
    res = run_bass_kernel_spmd(nc, in_maps, list(range(NCORES)))
    out = np.concatenate([untile_out(res.results[c]["out_t"])
                          for c in range(NCORES)], axis=0)
    return np.ascontiguousarray(out, dtype=np.float32)


# revision 13
# speedup vs baseline: 2.2500x; 1.9788x over previous
"""CTC aligner kernel for Trainium2 (8 NeuronCores, data-parallel over batch).

The reference's forward/backward recursions collapse: in each scan step the
logsumexp factors out (f_prev has no s-dependence in the added term), so with
q[t] = logits[t] + trans[t-1] (+b2), q[0] = f0, q[T] = 0:
    alignments[t] = softmax_s(q[t] + q[t+1])
The only heavy compute left is the pair MLP (trans), a batched matmul.

Per core (4 examples): frames are DMA-xbar-transposed (bf16) into [h, t]
layout, MM1 (frames @ W1, K=512 contracted as 4x128 over two shifted time
views) gives hidden^T in PSUM, relu+b1 writes hidden^T to SBUF bf16.

Time uses a group-inner mapping t = 16*p + g (partition p, group g 0..16).
MM2 computes trans[t-1] for group g with a stride-16 stationary slice of
hidden^T; a K=1 ones-matmul injects b2 and initializes PSUM. q[p, g] =
logits[t] + trans[t-1] + b2 then needs only a FREE-dim shift for
score[p, g] = q[p, g] + q[p, g+1] - no cross-partition traffic at all.
The f0 row and the q[T] = 0 cell are folded into host-prepared logits
(f0 - b2 at t=0, -b2 at t=2000), so there are no edge fixups on device.
Softmax: exp / per-group row-sum / reciprocal / scale.

logits_t[b, p, g, s] = logits[b, 16p + g, s] (g up to 16, overlapping) and
out_t[b, p, g, s] = out[b, 16p + g, s] keep every DMA contiguous per
partition; un-tiling on host is a plain reshape.
"""

import numpy as np
import ml_dtypes

B, T, S, H = 32, 2000, 63, 256
NCORES = 8
BPC = B // NCORES  # examples per core
TPAD = 2048        # t padded to 128*16
NG = 16            # groups: t = 16*p + g
HPAD = 2064        # hidden^T free width: 16*129 (stride-16 slices need +1)

_built = {}


def _build():
    from concourse import mybir, tile, bacc

    f32 = mybir.dt.float32
    bf16 = mybir.dt.bfloat16
    Alu = mybir.AluOpType
    Act = mybir.ActivationFunctionType

    nc = bacc.Bacc("TRN2", target_bir_lowering=False, debug=False,
                   num_devices=NCORES)

    frames_bf = nc.dram_tensor("frames_bf", [BPC, TPAD, H], bf16,
                               kind="ExternalInput").ap()
    # host-pretiled overlapping groups: logits_t[b, p, g, s], g in 0..16
    logits_t = nc.dram_tensor("logits_t", [BPC, 128, NG + 1, S], f32,
                              kind="ExternalInput").ap()
    w1 = nc.dram_tensor("W1", [2 * H, H], bf16, kind="ExternalInput").ap()
    w2 = nc.dram_tensor("W2", [H, S], bf16, kind="ExternalInput").ap()
    b1 = nc.dram_tensor("b1", [H, 1], f32, kind="ExternalInput").ap()
    b2r = nc.dram_tensor("b2r", [1, 8 * S], bf16, kind="ExternalInput").ap()
    # host un-tiles by reshape: out[b, 16p + g, s] = out_t[b, p, g, s]
    out_t = nc.dram_tensor("out_t", [BPC, 128, NG, S], f32,
                           kind="ExternalOutput").ap()

    # MM1 time chunks over the 1999 pair indices
    CHUNKS = [(0, 512), (512, 512), (1024, 512), (1536, 463)]

    with tile.TileContext(nc) as tc:
        with (
            tc.tile_pool(name="const", bufs=1) as constp,
            tc.tile_pool(name="frames", bufs=1) as framesp,
            tc.tile_pool(name="hid", bufs=1) as hidp,
            tc.tile_pool(name="mm1ps", bufs=3, space="PSUM") as mm1p,
            tc.tile_pool(name="mm2ps", bufs=3, space="PSUM") as mm2p,
            tc.tile_pool(name="mm2cps", bufs=2, space="PSUM") as mm2cp,
            tc.tile_pool(name="pw", bufs=3) as pwp,
        ):
            w1sb = constp.tile([128, 4, H], bf16)
            nc.sync.dma_start(w1sb[:], w1.rearrange("(k p) j -> p k j", p=128))
            w2sb = constp.tile([128, 2, S], bf16)
            nc.sync.dma_start(w2sb[:], w2.rearrange("(k p) s -> p k s", p=128))
            b1sb = constp.tile([128, 2], f32)
            nc.sync.dma_start(b1sb[:], b1.rearrange("(j p) o -> p (j o)", p=128))
            b2sb = constp.tile([1, 8 * S], bf16)
            nc.sync.dma_start(b2sb[:], b2r[:])
            ones = constp.tile([1, 128], bf16)
            nc.vector.memset(ones[:], 1.0)

            # ---- transpose-load all frames up front ----
            fts = []
            for b in range(BPC):
                ft = framesp.tile([128, 2, TPAD], bf16, tag=f"ft{b}")
                for h in range(2):
                    nc.sync.dma_start(ft[:, h],
                                      frames_bf[b, :, 128 * h:128 * h + 128],
                                      transpose=True)
                fts.append(ft)

            hids = []
            for b in range(BPC):
                hid = hidp.tile([128, 2, HPAD], bf16, tag=f"hid{b}")
                nc.gpsimd.memset(hid[:, :, 0:1], 0.0)
                nc.gpsimd.memset(hid[:, :, 2000:], 0.0)
                hids.append(hid)

            for b in range(BPC):
                ft, hid = fts[b], hids[b]
                # hidr[p, k, g, t16]: hid column 16*t16 + g
                hidr = hid[:].rearrange("p k (t g) -> p k g t", g=NG)
                # ---- MM1: hidden^T = relu(pairs @ W1 + b1) ----
                for ci, (t0, w) in enumerate(CHUNKS):
                    for j in range(2):
                        ps = mm1p.tile([128, 512], f32, tag="mm1")
                        for k in range(4):
                            half = k if k < 2 else k - 2
                            toff = t0 + 1 if k < 2 else t0
                            nc.tensor.matmul(
                                ps[:, :w],
                                w1sb[:, k, 128 * j:128 * j + 128],
                                ft[:, half, toff:toff + w],
                                start=(k == 0), stop=(k == 3),
                            )
                        dst = hid[:, j, 1 + t0:1 + t0 + w]
                        if (ci + j) % 2 == 0:
                            nc.vector.tensor_scalar(
                                dst, ps[:, :w], b1sb[:, j:j + 1], 0.0,
                                op0=Alu.add, op1=Alu.max)
                        else:
                            nc.scalar.activation(dst, ps[:, :w], Act.Relu,
                                                 bias=b1sb[:, j:j + 1])

                # ---- MM2 + q build: q[p, g] = L + trans[t-1] + b2 ----
                lt = pwp.tile([128, NG + 1, S], f32, tag="L")
                nc.sync.dma_start(lt[:], logits_t[b])
                q = pwp.tile([128, NG + 1, S], f32, tag="q")
                for st in range(2):
                    ps2 = mm2p.tile([128, 8, S], f32, tag="mm2")
                    ps2f = ps2[:].rearrange("p g s -> p (g s)")
                    nc.tensor.matmul(ps2f, ones[0:1, :], b2sb[:],
                                     start=True, stop=False)
                    for gl in range(8):
                        g = 8 * st + gl
                        for k in range(2):
                            nc.tensor.matmul(
                                ps2[:, gl],
                                hidr[:, k, g, 0:128],
                                w2sb[:, k],
                                start=False, stop=(gl == 7 and k == 1),
                            )
                    nc.vector.tensor_add(q[:, 8 * st:8 * st + 8],
                                         lt[:, 8 * st:8 * st + 8], ps2[:])
                # group 16 (t = 16p + 16): stationary slice starts at col 16,
                # same as group 0 shifted one t16 step
                ps2c = mm2cp.tile([128, S], f32, tag="mm2c")
                nc.tensor.matmul(ps2c[:], ones[0:1, :], b2sb[0:1, 0:S],
                                 start=True, stop=False)
                for k in range(2):
                    nc.tensor.matmul(ps2c[:], hidr[:, k, 0, 1:129],
                                     w2sb[:, k], start=False, stop=(k == 1))
                nc.vector.tensor_add(q[:, NG], lt[:, NG], ps2c[:])

                # ---- pointwise: score = q[:, g] + q[:, g+1] (free shift) ----
                sc = pwp.tile([128, NG, S], f32, tag="sc")
                nc.vector.tensor_add(sc[:], q[:, 0:NG], q[:, 1:NG + 1])
                e = pwp.tile([128, NG, S], f32, tag="e")
                nc.scalar.activation(e[:], sc[:], Act.Exp)
                sums = pwp.tile([128, NG], f32, tag="sums")
                nc.vector.reduce_sum(sums[:], e[:], mybir.AxisListType.X)
                rec = pwp.tile([128, NG], f32, tag="rec")
                nc.vector.reciprocal(rec[:], sums[:])
                r = pwp.tile([128, NG, S], f32, tag="r")
                nc.vector.tensor_mul(r[:, 0:8], e[:, 0:8],
                                     rec[:, 0:8].broadcast_to([128, 8, S]))
                for g in range(8, NG):
                    nc.scalar.mul(r[:, g], e[:, g], rec[:, g:g + 1])
                nc.sync.dma_start(out_t[b], r[:])

    nc.compile()
    return nc


def _get_nc():
    if "nc" not in _built:
        _built["nc"] = _build()
    return _built["nc"]


def prep_in_maps(ctc_logits, frame_embeddings, keyword_tokens, W1, b1, W2, b2):
    ctc_logits = np.asarray(ctc_logits, dtype=np.float32)
    frame_embeddings = np.asarray(frame_embeddings, dtype=np.float32)
    keyword_tokens = np.asarray(keyword_tokens)
    W1 = np.asarray(W1, dtype=np.float32)
    b1 = np.asarray(b1, dtype=np.float32)
    W2 = np.asarray(W2, dtype=np.float32)
    b2 = np.asarray(b2, dtype=np.float32)

    # f0[b, s] = (s == 0) * log_softmax(logits[b, 0])[kw[b, 0]]
    lg0 = ctc_logits[:, 0, :].astype(np.float64)
    m = lg0.max(axis=-1)
    lse = m + np.log(np.exp(lg0 - m[:, None]).sum(axis=-1))
    kw0 = keyword_tokens[:, 0].astype(np.int64)
    f0 = np.zeros((B, S), np.float32)
    f0[:, 0] = (lg0[np.arange(B), kw0] - lse).astype(np.float32)

    frames_bf = np.zeros((B, TPAD, H), ml_dtypes.bfloat16)
    frames_bf[:, :T] = frame_embeddings.astype(ml_dtypes.bfloat16)

    # logits_t[b, p, g, s] = logits[b, 16p + g, s], g in 0..16 (overlapping)
    logits_pad = np.zeros((B, TPAD, S), np.float32)
    logits_pad[:, :T] = ctc_logits
    base = logits_pad.reshape(B, 128, NG, S)
    col16 = np.zeros((B, 128, 1, S), np.float32)
    col16[:, :127, 0] = base[:, 1:, 0]
    logits_tl = np.concatenate([base, col16], axis=2)
    # fold the f0 row and the q[T]=0 cell into the logits:
    # q[0,0] = lt[0,0] + b2 (trans part is 0)  -> want f0
    logits_tl[:, 0, 0, :] = f0 - b2
    # q[124,16] is t=2000: lt + b2 + 0 -> want 0
    logits_tl[:, 124, 16, :] = -b2

    w1b = np.ascontiguousarray(W1).astype(ml_dtypes.bfloat16)
    w2b = np.ascontiguousarray(W2).astype(ml_dtypes.bfloat16)
    b1c = np.ascontiguousarray(b1.reshape(H, 1), dtype=np.float32)
    b2r = np.ascontiguousarray(np.tile(b2, 8).reshape(1, 8 * S)).astype(
        ml_dtypes.bfloat16)

    in_maps = []
    for c in range(NCORES):
        sl = slice(BPC * c, BPC * (c + 1))
        in_maps.append({
            "frames_bf": np.ascontiguousarray(frames_bf[sl]),
            "logits_t": np.ascontiguousarray(logits_tl[sl]),
            "W1": w1b, "W2": w2b, "b1": b1c, "b2r": b2r,
        })
    return in_maps


def untile_out(res_out):
    # res_out: (BPC, 128, NG, S), t = 16p + g -> plain reshape
    return res_out.reshape(BPC, TPAD, S)[:, :T, :]


def kernel(ctc_logits, frame_embeddings, keyword_tokens, W1, b1, W2, b2):
    from concourse.bass_utils import run_bass_kernel_spmd

    in_maps = prep_in_maps(ctc_logits, frame_embeddings, keyword_tokens,
                           W1, b1, W2, b2)
    nc = _get_nc()
    res = run_bass_kernel_spmd(nc, in_maps, list(range(NCORES)))
    out = np.concatenate([untile_out(res.results[c]["out_t"])
                          for c in range(NCORES)], axis=0)
    return np.ascontiguousarray(out, dtype=np.float32)
